# revision 26
# baseline (speedup 1.0000x reference)
"""CrossAttentionFusion Trainium2 kernel (nn_CrossAttentionFusion__45561013076033).

Full inputs -> full output. Sharding: 8 cores, core c handles batch b=c//2,
query-half h=c%2 (2048 of 4096 queries). Each core holds the full [256,4096]
cnn feature map of its batch (keys), its query-half of the transformer
features, and replicated weights.

Key restructurings vs the naive dataflow:
  * out = Wf1 @ x_trf + Wf2 @ attended + bf'.  Fold Wf2 into the value
    projection: U = (Wf2 @ Wv) @ x_cnn, so attention directly produces
    conv-ready channels; bv's contribution is constant (softmax rows sum
    to 1) and lands in bf' = bf + Wf2 @ bv.
  * Q/K projections and scores all run as fp8e4m3 DoubleRow matmuls
    (256-deep contraction in one pass).  Host ships x_trf/x_cnn in fp8
    and the projection weights in fp8 scaled by 8 (fp8-friendly range);
    the combined 1/(16*64) score scale is applied by the exp free affine.
  * Scores are computed pre-transposed, S^T[k, q] = K_kt^T Q, so the P^T
    needed by the PV matmul comes straight out of exp() -- no transpose
    of the [N, N] attention matrix.
  * Softmax row-sums come free as a 257th column of the PV matmul by
    augmenting U^T with a constant-16 column: [16A | 16R] = P [16U | 16]
    (16x keeps (Wf2 Wv) entries inside fp8 range; cancels in A/R).
  * Normalization (1/16R per query) is applied by the vector engine while
    moving the PV result PSUM->SBUF (per-partition scalar multiply,
    queries on partitions there).
  * The [q, e] -> [e, q] layout fix-up is a matmul with a 128x128 identity
    as the moving operand, accumulated directly into the Wf1 PSUM group
    (Wf1 runs in bf16: separate, pipelined weight loads).
"""

import numpy as np

B, C, H, W = 4, 256, 64, 64
N = H * W            # 4096 tokens
NCORES = 8
QH = N // 2          # 2048 queries per core
CT = C // 128        # 2 channel tiles
KC = N // 512        # 8 key chunks of 512
NSB = QH // 512      # 4 superblocks per core
NKT = N // 128       # 32 key tiles

_CACHE = {}


def _build():
    import concourse.bass as bass
    import concourse.mybir as mybir
    import concourse.tile as tile
    from concourse import bacc
    from concourse.masks import make_identity

    f32 = mybir.dt.float32
    bf16 = mybir.dt.bfloat16
    f16 = mybir.dt.float16
    f8 = mybir.dt.float8e4
    AF = mybir.ActivationFunctionType
    DR = mybir.MatmulPerfMode.DoubleRow

    nc = bacc.Bacc("TRN2", target_bir_lowering=False, debug=True)

    XQ8 = nc.dram_tensor("xq8", [C, QH], f8, kind="ExternalInput")
    XC8 = nc.dram_tensor("xc8", [C, N], f8, kind="ExternalInput")
    XQB = nc.dram_tensor("xqb", [C, QH], bf16, kind="ExternalInput")
    WQ8 = nc.dram_tensor("wq8", [C, C], f8, kind="ExternalInput")
    WK8 = nc.dram_tensor("wk8", [C, C], f8, kind="ExternalInput")
    WU8 = nc.dram_tensor("wu8", [C, C], f8, kind="ExternalInput")
    WF1 = nc.dram_tensor("wf1", [C, C], bf16, kind="ExternalInput")
    BIAS = nc.dram_tensor("bias3", [3, C], f32, kind="ExternalInput")
    OUT = nc.dram_tensor("out", [C, QH], f32, kind="ExternalOutput")

    xq8_d = XQ8.ap().rearrange("(t p) n -> p t n", p=128)
    xc8_d = XC8.ap().rearrange("(t p) n -> p t n", p=128)
    xqb_d = XQB.ap().rearrange("(t p) n -> p t n", p=128)
    wq_d = WQ8.ap().rearrange("(t p) d -> p t d", p=128)
    wk_d = WK8.ap().rearrange("(t p) d -> p t d", p=128)
    wu_d = WU8.ap().rearrange("(t p) d -> p t d", p=128)
    wf_d = WF1.ap().rearrange("(t p) d -> p t d", p=128)
    out_d = OUT.ap().rearrange("(t p) n -> p t n", p=128)

    with tile.TileContext(nc) as tc:
        with tc.tile_pool(name="persist", bufs=1) as per, \
             tc.tile_pool(name="pt", bufs=2) as ptp, \
             tc.tile_pool(name="cb", bufs=4) as cbp, \
             tc.tile_pool(name="outp", bufs=2) as outp, \
             tc.tile_pool(name="mm", bufs=2, space="PSUM") as mmp, \
             tc.tile_pool(name="pv", bufs=2, space="PSUM") as pvp, \
             tc.tile_pool(name="po", bufs=1, space="PSUM") as pop:

            # ---- persistent tiles ----
            xq8_sb = per.tile([128, CT, QH], f8)
            xc8_sb = per.tile([128, CT, N], f8)
            xqb_sb = per.tile([128, CT, QH], bf16)
            wq_sb = per.tile([128, CT, C], f8)
            wk_sb = per.tile([128, CT, C], f8)
            wu_sb = per.tile([128, CT, C], f8)
            wf_sb = per.tile([128, CT, C], bf16)
            bias_sb = per.tile([128, 3, CT], f32)
            q8_sb = per.tile([128, CT, QH], f8)
            k8_sb = per.tile([128, CT, N], f8)
            ut_sb = per.tile([128, NKT, C + 1], f16)
            ident = per.tile([128, 128], f16)

            # input DMAs, ordered by first consumer.  1024-column chunks
            # keep per-partition DMA rows >= 1KB (descriptor efficiency);
            # the three bias vectors ride in one packed tensor.
            nc.sync.dma_start(wq_sb[:], wq_d)
            nc.sync.dma_start(xq8_sb[:, :, 0:1024], xq8_d[:, :, 0:1024])
            nc.sync.dma_start(wk_sb[:], wk_d)
            nc.sync.dma_start(xc8_sb[:, :, 0:1024], xc8_d[:, :, 0:1024])
            nc.sync.dma_start(bias_sb[:],
                              BIAS.ap().rearrange("b (t p) -> p b t", p=128))
            nc.sync.dma_start(xq8_sb[:, :, 1024:2048], xq8_d[:, :, 1024:2048])
            nc.sync.dma_start(xc8_sb[:, :, 1024:2048], xc8_d[:, :, 1024:2048])
            nc.sync.dma_start(wu_sb[:], wu_d)
            nc.sync.dma_start(xc8_sb[:, :, 2048:3072], xc8_d[:, :, 2048:3072])
            nc.sync.dma_start(wf_sb[:], wf_d)
            nc.sync.dma_start(xc8_sb[:, :, 3072:4096], xc8_d[:, :, 3072:4096])
            for i in range(QH // 1024):
                s = slice(i * 1024, (i + 1) * 1024)
                nc.sync.dma_start(xqb_sb[:, :, s], xqb_d[:, :, s])
            make_identity(nc, ident[:])
            nc.gpsimd.memset(ut_sb[:, :, C:C + 1], 16.0)


            # ---- Q projection (fp8 DoubleRow) -> fp8, values are 8q ----
            for qc in range(QH // 512):
                s = slice(qc * 512, (qc + 1) * 512)
                for dt in range(CT):
                    ps = mmp.tile([128, 1024], f32, tag="mm")
                    nc.tensor.matmul(
                        ps[:, 0:512], wq_sb[:, :, dt * 128:(dt + 1) * 128],
                        xq8_sb[:, :, s], perf_mode=DR, start=True, stop=True)
                    nc.scalar.activation(q8_sb[:, dt, s], ps[:, 0:512],
                                         AF.Identity,
                                         bias=bias_sb[:, 0, dt:dt + 1])

            # ---- K projection (fp8 DoubleRow) -> fp8, values are 8k ----
            for kc in range(KC):
                s = slice(kc * 512, (kc + 1) * 512)
                for dt in range(CT):
                    ps = mmp.tile([128, 1024], f32, tag="mm")
                    nc.tensor.matmul(
                        ps[:, 0:512], wk_sb[:, :, dt * 128:(dt + 1) * 128],
                        xc8_sb[:, :, s], perf_mode=DR, start=True, stop=True)
                    nc.scalar.activation(k8_sb[:, dt, s], ps[:, 0:512],
                                         AF.Identity,
                                         bias=bias_sb[:, 1, dt:dt + 1])

            # score scale: S = (8q . 8k) = 64 s ; softmax wants s/16
            escale = 1.0 / (16.0 * 64.0)

            def emit_st_chunk(pt_sb, sb, g0, g1):
                """S^T = K_kt^T Q_sb (fp8 DoubleRow); P^T = exp(S^T/1024)"""
                qs = slice(sb * 512, (sb + 1) * 512)
                for g in range(g0, g1):
                    ps = mmp.tile([128, 1024], f32, tag="mm", name="ps")
                    for j in range(2):
                        kt = 2 * g + j
                        nc.tensor.matmul(
                            ps[:, j * 512:(j + 1) * 512],
                            k8_sb[:, :, kt * 128:(kt + 1) * 128],
                            q8_sb[:, :, qs], perf_mode=DR,
                            start=True, stop=True)
                    nc.scalar.activation(pt_sb[:, 2 * g:2 * g + 2], ps[:],
                                         AF.Exp, scale=escale)

            def new_pt():
                return ptp.tile([128, NKT, 512], f16, tag="pt", name="pt_sb")

            # ---- attention + fused conv, per 512-query superblock,
            # software-pipelined: S^T(sb+1) chunks are interleaved with
            # this sb's PV blocks so the tensor engine never waits on the
            # exp() backlog ----
            pt_cur = new_pt()
            emit_st_chunk(pt_cur, 0, 0, NKT // 2)

            # U^T = x_cnn^T (16 Wu)^T  [keys, 256], fp8 DoubleRow (emitted
            # after S^T(0): it needs the full x_cnn DMA, S^T only its head)
            for g in range(NKT // 2):
                ps = mmp.tile([128, 1024], f32, tag="mm", name="ps")
                for j in range(2):
                    mt = 2 * g + j
                    nc.tensor.matmul(
                        ps[:, j * 512:j * 512 + C],
                        xc8_sb[:, :, mt * 128:(mt + 1) * 128],
                        wu_sb[:], perf_mode=DR, start=True, stop=True)
                    nc.vector.tensor_copy(ut_sb[:, mt, :C],
                                          ps[:, j * 512:j * 512 + C])

            for sb in range(NSB):
                qs = slice(sb * 512, (sb + 1) * 512)
                pt_sb = pt_cur

                # conv part 1 into psO (bf16 weights: pipelined LDW)
                pso = [pop.tile([128, 512], f32, tag=f"po{et}", name=f"pso{et}")
                       for et in range(CT)]
                for et in range(CT):
                    for ct in range(CT):
                        nc.tensor.matmul(
                            pso[et][:], wf_sb[:, ct, et * 128:(et + 1) * 128],
                            xqb_sb[:, ct, qs],
                            start=(ct == 0), stop=False)

                pt_next = new_pt() if sb + 1 < NSB else None

                # PV: [16A | 16R] per 128-query block, normalize on DVE
                c_blk = []
                for qj in range(4):
                    if pt_next is not None:
                        emit_st_chunk(pt_next, sb + 1,
                                      qj * (NKT // 8), (qj + 1) * (NKT // 8))
                    psb = pvp.tile([128, C + 1], f32, tag="pv")
                    for kt in range(NKT):
                        nc.tensor.matmul(
                            psb[:], pt_sb[:, kt, qj * 128:(qj + 1) * 128],
                            ut_sb[:, kt],
                            start=(kt == 0), stop=(kt == NKT - 1))
                    rinv = cbp.tile([128, 1], f32, tag="rinv")
                    nc.vector.reciprocal(rinv[:], psb[:, C:C + 1])
                    c_sb = cbp.tile([128, C], f16, tag="c")
                    nc.vector.tensor_scalar_mul(c_sb[:], psb[:, :C], rinv[:])
                    c_blk.append(c_sb)

                # transpose each c block into the psO accumulation via
                # identity-matmul; last one closes the group
                for qj in range(4):
                    for et in range(CT):
                        nc.tensor.matmul(
                            pso[et][:, qj * 128:(qj + 1) * 128],
                            c_blk[qj][:, et * 128:(et + 1) * 128],
                            ident[:],
                            start=False, stop=(qj == 3),
                            skip_group_check=True)

                # final combine + bias on the vector engine (keeps psO
                # recycling off the scalar engine's exp backlog)
                for et in range(CT):
                    o_sb = outp.tile([128, 512], f32, tag="o")
                    nc.vector.tensor_scalar_add(o_sb[:], pso[et][:],
                                                bias_sb[:, 2, et:et + 1])
                    nc.sync.dma_start(out_d[:, et, qs], o_sb[:])
                pt_cur = pt_next
    nc.finalize()
    return nc


def _get_nc():
    if "nc" not in _CACHE:
        _CACHE["nc"] = _build()
    return _CACHE["nc"]


def _in_maps(transformer_features, cnn_features, Wq, bq, Wk, bk, Wv, bv, Wf, bf):
    import ml_dtypes
    f8 = ml_dtypes.float8_e4m3fn

    xt = np.ascontiguousarray(np.asarray(transformer_features, np.float32)
                              .reshape(B, C, N))
    xc = np.ascontiguousarray(np.asarray(cnn_features, np.float32)
                              .reshape(B, C, N))
    Wq = np.asarray(Wq, np.float32)
    Wk = np.asarray(Wk, np.float32)
    Wv = np.asarray(Wv, np.float32)
    Wf = np.asarray(Wf, np.float32)
    bq = np.asarray(bq, np.float32)
    bk = np.asarray(bk, np.float32)
    bv = np.asarray(bv, np.float32)
    bf = np.asarray(bf, np.float32)

    Wf1, Wf2 = Wf[:, :C], Wf[:, C:]
    wq8 = np.ascontiguousarray(8.0 * Wq.T).astype(f8)
    wk8 = np.ascontiguousarray(8.0 * Wk.T).astype(f8)
    wu8 = np.ascontiguousarray(16.0 * (Wf2 @ Wv).T).astype(f8)
    wf1 = np.ascontiguousarray(Wf1.T).astype(ml_dtypes.bfloat16)
    bias3 = np.ascontiguousarray(
        np.stack([8.0 * bq, 8.0 * bk, bf + Wf2 @ bv]))
    xc8 = xc.astype(f8)

    maps = []
    for c in range(NCORES):
        b, h = divmod(c, 2)
        xq = np.ascontiguousarray(xt[b][:, h * QH:(h + 1) * QH])
        maps.append(dict(
            xq8=xq.astype(f8),
            xc8=xc8[b],
            xqb=xq.astype(ml_dtypes.bfloat16),
            wq8=wq8, wk8=wk8, wu8=wu8, wf1=wf1,
            bias3=bias3,
        ))
    return maps


def _run(inputs, trace=False):
    from concourse.bass_utils import run_bass_kernel_spmd
    nc = _get_nc()
    maps = _in_maps(**inputs)
    return run_bass_kernel_spmd(nc, maps, list(range(NCORES)), trace=trace)


def kernel(**inputs) -> np.ndarray:
    res = _run(inputs).results
    out = np.empty((B, C, N), np.float32)
    for c in range(NCORES):
        b, h = divmod(c, 2)
        out[b][:, h * QH:(h + 1) * QH] = res[c]["out"]
    return out.reshape(B, C, H, W)


# revision 30
# speedup vs baseline: 1.0158x; 1.0158x over previous
"""CrossAttentionFusion Trainium2 kernel (nn_CrossAttentionFusion__45561013076033).

Full inputs -> full output. Sharding: 8 cores, core c handles batch b=c//2,
query-half h=c%2 (2048 of 4096 queries). Each core holds the full [256,4096]
cnn feature map of its batch (keys), its query-half of the transformer
features, and replicated weights.

Key restructurings vs the naive dataflow:
  * out = Wf1 @ x_trf + Wf2 @ attended + bf'.  Fold Wf2 into the value
    projection: U = (Wf2 @ Wv) @ x_cnn, so attention directly produces
    conv-ready channels; bv's contribution is constant (softmax rows sum
    to 1) and lands in bf' = bf + Wf2 @ bv.
  * Q/K projections and scores all run as fp8e4m3 DoubleRow matmuls
    (256-deep contraction in one pass).  Host ships x_trf/x_cnn in fp8
    and the projection weights in fp8 scaled by 8 (fp8-friendly range);
    the combined 1/(16*64) score scale is applied by the exp free affine.
  * Scores are computed pre-transposed, S^T[k, q] = K_kt^T Q, so the P^T
    needed by the PV matmul comes straight out of exp() -- no transpose
    of the [N, N] attention matrix.
  * Softmax row-sums come free as a 257th column of the PV matmul by
    augmenting U^T with a constant-16 column: [16A | 16R] = P [16U | 16]
    (16x keeps (Wf2 Wv) entries inside fp8 range; cancels in A/R).
  * Normalization (1/16R per query) is applied by the vector engine while
    moving the PV result PSUM->SBUF (per-partition scalar multiply,
    queries on partitions there).
  * The [q, e] -> [e, q] layout fix-up is a matmul with a 128x128 identity
    as the moving operand, accumulated directly into the Wf1 PSUM group
    (Wf1 runs in bf16: separate, pipelined weight loads).
"""

import numpy as np

B, C, H, W = 4, 256, 64, 64
N = H * W            # 4096 tokens
NCORES = 8
QH = N // 2          # 2048 queries per core
CT = C // 128        # 2 channel tiles
KC = N // 512        # 8 key chunks of 512
NSB = QH // 512      # 4 superblocks per core
NKT = N // 128       # 32 key tiles

_CACHE = {}


def _build():
    import concourse.bass as bass
    import concourse.mybir as mybir
    import concourse.tile as tile
    from concourse import bacc
    from concourse.masks import make_identity

    f32 = mybir.dt.float32
    bf16 = mybir.dt.bfloat16
    f16 = mybir.dt.float16
    f8 = mybir.dt.float8e4
    AF = mybir.ActivationFunctionType
    DR = mybir.MatmulPerfMode.DoubleRow

    nc = bacc.Bacc("TRN2", target_bir_lowering=False, debug=True)

    XQ8 = nc.dram_tensor("xq8", [C, QH], f8, kind="ExternalInput")
    XC8 = nc.dram_tensor("xc8", [C, N], f8, kind="ExternalInput")
    XQB = nc.dram_tensor("xqb", [C, QH], bf16, kind="ExternalInput")
    WQ8 = nc.dram_tensor("wq8", [C, C], f8, kind="ExternalInput")
    WK8 = nc.dram_tensor("wk8", [C, C], f8, kind="ExternalInput")
    WU8 = nc.dram_tensor("wu8", [C, C], f8, kind="ExternalInput")
    WF1 = nc.dram_tensor("wf1", [C, C], bf16, kind="ExternalInput")
    BIAS = nc.dram_tensor("bias3", [3, C], f32, kind="ExternalInput")
    OUT = nc.dram_tensor("out", [C, QH], f32, kind="ExternalOutput")

    xq8_d = XQ8.ap().rearrange("(t p) n -> p t n", p=128)
    xc8_d = XC8.ap().rearrange("(t p) n -> p t n", p=128)
    xqb_d = XQB.ap().rearrange("(t p) n -> p t n", p=128)
    wq_d = WQ8.ap().rearrange("(t p) d -> p t d", p=128)
    wk_d = WK8.ap().rearrange("(t p) d -> p t d", p=128)
    wu_d = WU8.ap().rearrange("(t p) d -> p t d", p=128)
    wf_d = WF1.ap().rearrange("(t p) d -> p t d", p=128)
    out_d = OUT.ap().rearrange("(t p) n -> p t n", p=128)

    with tile.TileContext(nc) as tc:
        with tc.tile_pool(name="persist", bufs=1) as per, \
             tc.tile_pool(name="pt", bufs=2) as ptp, \
             tc.tile_pool(name="cb", bufs=4) as cbp, \
             tc.tile_pool(name="outp", bufs=2) as outp, \
             tc.tile_pool(name="mm", bufs=2, space="PSUM") as mmp, \
             tc.tile_pool(name="pv", bufs=2, space="PSUM") as pvp, \
             tc.tile_pool(name="po", bufs=1, space="PSUM") as pop:

            # ---- persistent tiles ----
            xq8_sb = per.tile([128, CT, QH], f8)
            xc8_sb = per.tile([128, CT, N], f8)
            xqb_sb = per.tile([128, CT, QH], bf16)
            wq_sb = per.tile([128, CT, C], f8)
            wk_sb = per.tile([128, CT, C], f8)
            wu_sb = per.tile([128, CT, C], f8)
            wf_sb = per.tile([128, CT, C], bf16)
            bias_sb = per.tile([128, 3, CT], f32)
            q8_sb = per.tile([128, CT, QH], f8)
            k8_sb = per.tile([128, CT, N], f8)
            ut_sb = per.tile([128, NKT, C + 1], f16)
            ident = per.tile([128, 128], f16)
            warm_sb = per.tile([1, 1], f32)

            # input DMAs, ordered by first consumer.  1024-column chunks
            # keep per-partition DMA rows >= 1KB (descriptor efficiency);
            # the three bias vectors ride in one packed tensor.
            # preload the exp activation table while the DMAs run (the
            # first real exp would otherwise pay the ~2.7us table switch)
            nc.vector.memset(warm_sb[:], 0.0)
            nc.scalar.activation(warm_sb[:], warm_sb[:], AF.Exp, scale=0.0)

            nc.sync.dma_start(wq_sb[:], wq_d)
            nc.sync.dma_start(xq8_sb[:, :, 0:512], xq8_d[:, :, 0:512])
            nc.sync.dma_start(wk_sb[:], wk_d)
            nc.sync.dma_start(xc8_sb[:, :, 0:512], xc8_d[:, :, 0:512])
            nc.sync.dma_start(xq8_sb[:, :, 512:1024], xq8_d[:, :, 512:1024])
            nc.sync.dma_start(xc8_sb[:, :, 512:1024], xc8_d[:, :, 512:1024])
            nc.sync.dma_start(bias_sb[:],
                              BIAS.ap().rearrange("b (t p) -> p b t", p=128))
            nc.sync.dma_start(xq8_sb[:, :, 1024:2048], xq8_d[:, :, 1024:2048])
            nc.sync.dma_start(xc8_sb[:, :, 1024:2048], xc8_d[:, :, 1024:2048])
            nc.sync.dma_start(wu_sb[:], wu_d)
            nc.sync.dma_start(xc8_sb[:, :, 2048:3072], xc8_d[:, :, 2048:3072])
            nc.sync.dma_start(wf_sb[:], wf_d)
            nc.sync.dma_start(xc8_sb[:, :, 3072:4096], xc8_d[:, :, 3072:4096])
            for i in range(QH // 1024):
                s = slice(i * 1024, (i + 1) * 1024)
                nc.sync.dma_start(xqb_sb[:, :, s], xqb_d[:, :, s])
            make_identity(nc, ident[:])
            nc.gpsimd.memset(ut_sb[:, :, C:C + 1], 16.0)


            # ---- Q projection (fp8 DoubleRow) -> fp8, values are 8q ----
            for qc in range(QH // 512):
                s = slice(qc * 512, (qc + 1) * 512)
                for dt in range(CT):
                    ps = mmp.tile([128, 1024], f32, tag="mm")
                    nc.tensor.matmul(
                        ps[:, 0:512], wq_sb[:, :, dt * 128:(dt + 1) * 128],
                        xq8_sb[:, :, s], perf_mode=DR, start=True, stop=True)
                    nc.scalar.activation(q8_sb[:, dt, s], ps[:, 0:512],
                                         AF.Identity,
                                         bias=bias_sb[:, 0, dt:dt + 1])

            # ---- K projection (fp8 DoubleRow) -> fp8, values are 8k ----
            for kc in range(KC):
                s = slice(kc * 512, (kc + 1) * 512)
                for dt in range(CT):
                    ps = mmp.tile([128, 1024], f32, tag="mm")
                    nc.tensor.matmul(
                        ps[:, 0:512], wk_sb[:, :, dt * 128:(dt + 1) * 128],
                        xc8_sb[:, :, s], perf_mode=DR, start=True, stop=True)
                    nc.scalar.activation(k8_sb[:, dt, s], ps[:, 0:512],
                                         AF.Identity,
                                         bias=bias_sb[:, 1, dt:dt + 1])

            # score scale: S = (8q . 8k) = 64 s ; softmax wants s/16
            escale = 1.0 / (16.0 * 64.0)

            def emit_st_chunk(pt_sb, sb, g0, g1):
                """S^T = K_kt^T Q_sb (fp8 DoubleRow); P^T = exp(S^T/1024)"""
                qs = slice(sb * 512, (sb + 1) * 512)
                for g in range(g0, g1):
                    ps = mmp.tile([128, 1024], f32, tag="mm", name="ps")
                    for j in range(2):
                        kt = 2 * g + j
                        nc.tensor.matmul(
                            ps[:, j * 512:(j + 1) * 512],
                            k8_sb[:, :, kt * 128:(kt + 1) * 128],
                            q8_sb[:, :, qs], perf_mode=DR,
                            start=True, stop=True)
                    nc.scalar.activation(pt_sb[:, 2 * g:2 * g + 2], ps[:],
                                         AF.Exp, scale=escale)

            def new_pt():
                return ptp.tile([128, NKT, 512], f16, tag="pt", name="pt_sb")

            # ---- attention + fused conv, per 512-query superblock,
            # software-pipelined: S^T(sb+1) chunks are interleaved with
            # this sb's PV blocks so the tensor engine never waits on the
            # exp() backlog.  U^T = x_cnn^T (16 Wu)^T [keys, 256] (fp8
            # DoubleRow) pairs are interleaved into S^T(0) for the same
            # reason (their PSUM is drained by DVE, not the scalar engine).
            pt_cur = new_pt()
            for g in range(NKT // 2):
                emit_st_chunk(pt_cur, 0, g, g + 1)
                ps = mmp.tile([128, 1024], f32, tag="mm", name="ps")
                for j in range(2):
                    mt = 2 * g + j
                    nc.tensor.matmul(
                        ps[:, j * 512:j * 512 + C],
                        xc8_sb[:, :, mt * 128:(mt + 1) * 128],
                        wu_sb[:], perf_mode=DR, start=True, stop=True)
                    nc.vector.tensor_copy(ut_sb[:, mt, :C],
                                          ps[:, j * 512:j * 512 + C])

            for sb in range(NSB):
                qs = slice(sb * 512, (sb + 1) * 512)
                pt_sb = pt_cur

                # conv part 1 into psO (bf16 weights: pipelined LDW)
                pso = [pop.tile([128, 512], f32, tag=f"po{et}", name=f"pso{et}")
                       for et in range(CT)]
                for et in range(CT):
                    for ct in range(CT):
                        nc.tensor.matmul(
                            pso[et][:], wf_sb[:, ct, et * 128:(et + 1) * 128],
                            xqb_sb[:, ct, qs],
                            start=(ct == 0), stop=False)

                pt_next = new_pt() if sb + 1 < NSB else None

                # PV: [16A | 16R] per 128-query block, normalize on DVE
                c_blk = []
                for qj in range(4):
                    if pt_next is not None:
                        emit_st_chunk(pt_next, sb + 1,
                                      qj * (NKT // 8), (qj + 1) * (NKT // 8))
                    psb = pvp.tile([128, C + 1], f32, tag="pv")
                    for kt in range(NKT):
                        nc.tensor.matmul(
                            psb[:], pt_sb[:, kt, qj * 128:(qj + 1) * 128],
                            ut_sb[:, kt],
                            start=(kt == 0), stop=(kt == NKT - 1))
                    rinv = cbp.tile([128, 1], f32, tag="rinv")
                    nc.vector.reciprocal(rinv[:], psb[:, C:C + 1])
                    c_sb = cbp.tile([128, C], f16, tag="c")
                    nc.vector.tensor_scalar_mul(c_sb[:], psb[:, :C], rinv[:])
                    c_blk.append(c_sb)

                # transpose each c block into the psO accumulation via
                # identity-matmul; last one closes the group
                for qj in range(4):
                    for et in range(CT):
                        nc.tensor.matmul(
                            pso[et][:, qj * 128:(qj + 1) * 128],
                            c_blk[qj][:, et * 128:(et + 1) * 128],
                            ident[:],
                            start=False, stop=(qj == 3),
                            skip_group_check=True)

                # final combine + bias on the vector engine (keeps psO
                # recycling off the scalar engine's exp backlog)
                for et in range(CT):
                    o_sb = outp.tile([128, 512], f32, tag="o")
                    nc.vector.tensor_scalar_add(o_sb[:], pso[et][:],
                                                bias_sb[:, 2, et:et + 1])
                    nc.sync.dma_start(out_d[:, et, qs], o_sb[:])
                pt_cur = pt_next
    nc.finalize()
    return nc


def _get_nc():
    if "nc" not in _CACHE:
        _CACHE["nc"] = _build()
    return _CACHE["nc"]


def _in_maps(transformer_features, cnn_features, Wq, bq, Wk, bk, Wv, bv, Wf, bf):
    import ml_dtypes
    f8 = ml_dtypes.float8_e4m3fn

    xt = np.ascontiguousarray(np.asarray(transformer_features, np.float32)
                              .reshape(B, C, N))
    xc = np.ascontiguousarray(np.asarray(cnn_features, np.float32)
                              .reshape(B, C, N))
    Wq = np.asarray(Wq, np.float32)
    Wk = np.asarray(Wk, np.float32)
    Wv = np.asarray(Wv, np.float32)
    Wf = np.asarray(Wf, np.float32)
    bq = np.asarray(bq, np.float32)
    bk = np.asarray(bk, np.float32)
    bv = np.asarray(bv, np.float32)
    bf = np.asarray(bf, np.float32)

    Wf1, Wf2 = Wf[:, :C], Wf[:, C:]
    wq8 = np.ascontiguousarray(8.0 * Wq.T).astype(f8)
    wk8 = np.ascontiguousarray(8.0 * Wk.T).astype(f8)
    wu8 = np.ascontiguousarray(16.0 * (Wf2 @ Wv).T).astype(f8)
    wf1 = np.ascontiguousarray(Wf1.T).astype(ml_dtypes.bfloat16)
    bias3 = np.ascontiguousarray(
        np.stack([8.0 * bq, 8.0 * bk, bf + Wf2 @ bv]))
    xc8 = xc.astype(f8)

    maps = []
    for c in range(NCORES):
        b, h = divmod(c, 2)
        xq = np.ascontiguousarray(xt[b][:, h * QH:(h + 1) * QH])
        maps.append(dict(
            xq8=xq.astype(f8),
            xc8=xc8[b],
            xqb=xq.astype(ml_dtypes.bfloat16),
            wq8=wq8, wk8=wk8, wu8=wu8, wf1=wf1,
            bias3=bias3,
        ))
    return maps


def _run(inputs, trace=False):
    from concourse.bass_utils import run_bass_kernel_spmd
    nc = _get_nc()
    maps = _in_maps(**inputs)
    return run_bass_kernel_spmd(nc, maps, list(range(NCORES)), trace=trace)


def kernel(**inputs) -> np.ndarray:
    res = _run(inputs).results
    out = np.empty((B, C, N), np.float32)
    for c in range(NCORES):
        b, h = divmod(c, 2)
        out[b][:, h * QH:(h + 1) * QH] = res[c]["out"]
    return out.reshape(B, C, H, W)
